# revision 10
# baseline (speedup 1.0000x reference)
"""Trainium2 Bass kernel for AdaptiveNet MLP (fc1+sigmoid, grouped fc2+sigmoid, fc3).

Sharding: pure data-parallel over batch across 8 NeuronCores (no collectives).
Each core computes its 2048-row shard through all three layers.

fc1 (95% of FLOPs) runs in fp8-e4m3 with DoubleRow perf mode (two fp8 weights
per PE cell -> K=256 per matmul, halving the matmul count); the sigmoid damps
the quantization error so the final rel-err stays ~3e-3 (gate is 2e-2).

Layout trick: H1 is permuted s-major on the host (h1' = s*512 + g, where the
original h1 = g*8 + s).  fc1 then produces hT' tiles [128 h1' partitions x 512
rows]; the grouped fc2 contraction over s becomes 8 fused multiply-accumulate
ops on the vector engine with per-partition scalars (W2 columns), and fc3 is a
plain bf16 matmul over the 512 groups.  Biases are per-partition [128,1]
columns fused into ScalarE sigmoids / a VectorE add.
"""

import sys

for _p in ("/opt/trn_rl_repo",):
    if _p not in sys.path:
        sys.path.append(_p)

import numpy as np
import ml_dtypes

BF16 = ml_dtypes.bfloat16
FP8 = ml_dtypes.float8_e4m3  # == mybir.dt.float8e4

D_IN, H1, H2, D_OUT = 1024, 4096, 512, 256
GS = H1 // H2  # 8
B = 16384
N_CORES = 8
B_SHARD = B // N_CORES  # 2048
NBLK = 512  # rows per block (one PSUM bank of fp32)
NB = B_SHARD // NBLK  # 4
KC = D_IN // 128  # 8 contraction subtiles for fc1
KP = KC // 2  # 4 DoubleRow pairs
CC = H1 // 128  # 32 h1' chunks
NT = H2 // 128  # 4 x2T tiles
ND = D_OUT // 128  # 2 output chunks

_compiled = {}


def _build_nc():
    from concourse import bacc, tile, mybir

    f32 = mybir.dt.float32
    bf16 = mybir.dt.bfloat16
    fp8 = mybir.dt.float8e4
    AF = mybir.ActivationFunctionType
    ALU = mybir.AluOpType
    DR = mybir.MatmulPerfMode.DoubleRow

    nc = bacc.Bacc("TRN2", target_bir_lowering=False, debug=False,
                   num_devices=N_CORES)

    xq = nc.dram_tensor("xq", [128, KC, B_SHARD], fp8, kind="ExternalInput")
    w1q = nc.dram_tensor("w1q", [128, KC, H1], fp8, kind="ExternalInput")
    w2c = nc.dram_tensor("w2c", [128, CC], f32, kind="ExternalInput")
    b1c = nc.dram_tensor("b1c", [128, CC], f32, kind="ExternalInput")
    b2c = nc.dram_tensor("b2c", [128, NT], f32, kind="ExternalInput")
    w3t = nc.dram_tensor("w3t", [H2, D_OUT], bf16, kind="ExternalInput")
    b3c = nc.dram_tensor("b3c", [128, ND], f32, kind="ExternalInput")
    out = nc.dram_tensor("out", [D_OUT, B_SHARD], f32, kind="ExternalOutput")

    with tile.TileContext(nc) as tc:
        with (
            tc.tile_pool(name="wpool", bufs=1) as wpool,
            tc.tile_pool(name="xpool", bufs=1) as xpool,
            tc.tile_pool(name="hpool", bufs=8) as hpool,
            tc.tile_pool(name="accpool", bufs=1) as accpool,
            tc.tile_pool(name="x2pool", bufs=1) as x2pool,
            tc.tile_pool(name="opool", bufs=4) as opool,
            tc.tile_pool(name="psum_h", bufs=8, space="PSUM") as psum_h_pool,
        ):
            psum_o_pool = psum_h_pool
            # --- constants ---
            w2_sb = wpool.tile([128, CC], f32, tag="w2c")
            nc.sync.dma_start(w2_sb[:], w2c.ap()[:])
            b1_sb = wpool.tile([128, CC], f32, tag="b1c")
            nc.sync.dma_start(b1_sb[:], b1c.ap()[:])
            b2_sb = wpool.tile([128, NT], f32, tag="b2c")
            nc.sync.dma_start(b2_sb[:], b2c.ap()[:])
            b3_sb = wpool.tile([128, ND], f32, tag="b3c")
            nc.sync.dma_start(b3_sb[:], b3c.ap()[:])

            # --- fc1 weights / inputs. DMA order matters for the ramp: the
            # first matmuls need only x pair 0 and the first H1 columns of
            # W1 pair 0, so land those first in small pieces. ---
            w1_sb = [None] * KP
            x_sb = [None] * NB
            w1_sb[0] = wpool.tile([128, 2, H1], fp8, tag="w1_0",
                                  name="w1sb_0")
            # first c-chunks of pair 0 in H1 quarters
            for q in range(4):
                nc.sync.dma_start(
                    w1_sb[0][:, :, q * (H1 // 4):(q + 1) * (H1 // 4)],
                    w1q.ap()[:, 0:2, q * (H1 // 4):(q + 1) * (H1 // 4)])
                if q == 0:
                    for n in range(NB):
                        x_sb[n] = xpool.tile([128, KC, NBLK], fp8,
                                             tag=f"x_{n}", name=f"xsb_{n}")
                        nc.sync.dma_start(
                            x_sb[n][:, 0:2, :],
                            xq.ap()[:, 0:2, n * NBLK:(n + 1) * NBLK])
            for j in range(1, KP):
                w1_sb[j] = wpool.tile([128, 2, H1], fp8, tag=f"w1_{j}",
                                      name=f"w1sb_{j}")
                nc.sync.dma_start(w1_sb[j][:],
                                  w1q.ap()[:, 2 * j:2 * j + 2, :])
                for n in range(NB):
                    nc.sync.dma_start(
                        x_sb[n][:, 2 * j:2 * j + 2, :],
                        xq.ap()[:, 2 * j:2 * j + 2,
                                n * NBLK:(n + 1) * NBLK])
            w3_sb = []
            for t_i in range(NT):
                t = wpool.tile([128, D_OUT], bf16, tag=f"w3_{t_i}",
                               name=f"w3sb_{t_i}")
                nc.sync.dma_start(t[:], w3t.ap()[128 * t_i:128 * (t_i + 1), :])
                w3_sb.append(t)

            # fc2 accumulators, one per (row-block, x2 tile)
            acc = [[None] * NT for _ in range(NB)]

            def fc2_step(c, n, ht):
                t_i = c % NT
                if c < NT:
                    acc[n][t_i] = accpool.tile([128, NBLK], bf16,
                                               tag=f"acc_{n}_{t_i}",
                                               name=f"acc_{n}_{t_i}")
                    nc.vector.tensor_scalar_mul(acc[n][t_i][:], ht[:],
                                                w2_sb[:, c:c + 1])
                else:
                    nc.vector.scalar_tensor_tensor(
                        acc[n][t_i][:], ht[:], w2_sb[:, c:c + 1],
                        acc[n][t_i][:], op0=ALU.mult, op1=ALU.add)

            def epilogue(c, ph):
                for n in range(NB):
                    ht = hpool.tile([128, NBLK], bf16, tag="ht",
                                    name=f"ht_{n}_{c}")
                    nc.scalar.activation(ht[:], ph[n][:], AF.Sigmoid,
                                         bias=b1_sb[:, c:c + 1])
                    fc2_step(c, n, ht)

            # --- fc1 + fc2 ---
            # Ramp phase: first 8 c-chunks iterate j-OUTER so the PE only
            # ever needs the W1/x pair that has already DMA'd in, never
            # stalling on the tail of the 6MB weight/input stream.
            RAMP = 8
            ph_ramp = [None] * RAMP
            for c in range(RAMP):
                ph_ramp[c] = [None] * NB
                for n in range(NB):
                    ph_ramp[c][n] = psum_h_pool.tile(
                        [128, NBLK], f32, tag="psum_h", name=f"ph_{n}_{c}")
            for j in range(KP):
                for c in range(RAMP):
                    for n in range(NB):
                        nc.tensor.matmul(
                            ph_ramp[c][n][:],
                            lhsT=w1_sb[j][:, :, 128 * c:128 * (c + 1)],
                            rhs=x_sb[n][:, 2 * j:2 * j + 2, :],
                            start=(j == 0),
                            stop=(j == KP - 1),
                            perf_mode=DR,
                        )
            for c in range(RAMP):
                epilogue(c, ph_ramp[c])

            # Steady phase: c-outer, all weights resident.
            for c in range(RAMP, CC):
                ph = [None] * NB
                for n in range(NB):
                    ph[n] = psum_h_pool.tile([128, NBLK], f32, tag="psum_h",
                                             name=f"ph_{n}_{c}")
                for j in range(KP):
                    for n in range(NB):
                        nc.tensor.matmul(
                            ph[n][:],
                            lhsT=w1_sb[j][:, :, 128 * c:128 * (c + 1)],
                            rhs=x_sb[n][:, 2 * j:2 * j + 2, :],
                            start=(j == 0),
                            stop=(j == KP - 1),
                            perf_mode=DR,
                        )
                epilogue(c, ph)

            # --- fc2 sigmoid + fc3 ---
            x2_sb = [[None] * NT for _ in range(NB)]
            for n in range(NB):
                for t_i in range(NT):
                    t = x2pool.tile([128, NBLK], bf16, tag=f"x2_{n}_{t_i}",
                                    name=f"x2sb_{n}_{t_i}")
                    nc.scalar.activation(t[:], acc[n][t_i][:], AF.Sigmoid,
                                         bias=b2_sb[:, t_i:t_i + 1])
                    x2_sb[n][t_i] = t

            for n in range(NB):
                for d in range(ND):
                    po = psum_o_pool.tile([128, NBLK], f32, tag="psum_h",
                                          name=f"po_{n}_{d}")
                    for t_i in range(NT):
                        nc.tensor.matmul(
                            po[:],
                            lhsT=w3_sb[t_i][:, 128 * d:128 * (d + 1)],
                            rhs=x2_sb[n][t_i][:],
                            start=(t_i == 0),
                            stop=(t_i == NT - 1),
                        )
                    ot = opool.tile([128, NBLK], f32, tag="ot",
                                    name=f"ot_{n}_{d}")
                    nc.vector.tensor_scalar_add(ot[:], po[:],
                                                b3_sb[:, d:d + 1])
                    nc.sync.dma_start(
                        out.ap()[128 * d:128 * (d + 1),
                                 n * NBLK:(n + 1) * NBLK], ot[:])

    nc.compile()
    return nc


def get_nc():
    if "nc" not in _compiled:
        _compiled["nc"] = _build_nc()
    return _compiled["nc"]


def make_in_maps(x, W1, b1, W2, b2, W3, b3):
    x = np.asarray(x, dtype=np.float32)
    W1 = np.asarray(W1, dtype=np.float32)
    b1 = np.asarray(b1, dtype=np.float32)
    W2 = np.asarray(W2, dtype=np.float32)
    b2 = np.asarray(b2, dtype=np.float32)
    W3 = np.asarray(W3, dtype=np.float32)
    b3 = np.asarray(b3, dtype=np.float32)

    # s-major permutation of H1: new index p = s*H2 + g  (old h1 = g*GS + s)
    p = np.arange(H1)
    perm = (p % H2) * GS + (p // H2)
    W1p = W1[perm, :]
    b1p = b1[perm]

    # fp8 fc1 operands in DoubleRow layout [128, KC, *]:
    # element (p, j, m) holds contraction index k = 128*j + p
    w1t = W1p.T.astype(FP8)  # [D_IN, H1]
    w1q_h = np.ascontiguousarray(
        w1t.reshape(KC, 128, H1).transpose(1, 0, 2))
    xt = x.T.astype(FP8)  # [D_IN, B]
    xq_h = np.ascontiguousarray(
        xt.reshape(KC, 128, B).transpose(1, 0, 2))

    b1c_h = np.ascontiguousarray(b1p.reshape(CC, 128).T, dtype=np.float32)
    # chunk c: s = c//NT, tile t = c%NT, partition k <-> group 128*t + k
    w2c_h = np.empty((128, CC), dtype=np.float32)
    for c in range(CC):
        w2c_h[:, c] = W2[128 * (c % NT):128 * (c % NT) + 128, c // NT]
    b2c_h = np.ascontiguousarray(b2.reshape(NT, 128).T, dtype=np.float32)
    w3t_h = np.ascontiguousarray(W3.T).astype(BF16)  # [H2, D_OUT]
    b3c_h = np.ascontiguousarray(b3.reshape(ND, 128).T, dtype=np.float32)

    in_maps = []
    for i in range(N_CORES):
        in_maps.append({
            "xq": np.ascontiguousarray(
                xq_h[:, :, i * B_SHARD:(i + 1) * B_SHARD]),
            "w1q": w1q_h,
            "w2c": w2c_h,
            "b1c": b1c_h,
            "b2c": b2c_h,
            "w3t": w3t_h,
            "b3c": b3c_h,
        })
    return in_maps


def kernel(x, W1, b1, W2, b2, W3, b3):
    from concourse.bass_utils import run_bass_kernel_spmd

    nc = get_nc()
    in_maps = make_in_maps(x, W1, b1, W2, b2, W3, b3)
    res = run_bass_kernel_spmd(nc, in_maps, core_ids=list(range(N_CORES)))
    outT = np.concatenate([res.results[i]["out"] for i in range(N_CORES)],
                          axis=1)  # [D_OUT, B]
    return np.ascontiguousarray(outT.T)


# revision 12
# speedup vs baseline: 1.1533x; 1.1533x over previous
"""Trainium2 Bass kernel for AdaptiveNet MLP (fc1+sigmoid, grouped fc2+sigmoid, fc3).

Sharding: pure data-parallel over batch across 8 NeuronCores (no collectives).
Each core computes its 2048-row shard through all three layers.

fc1 (95% of FLOPs) runs in fp8-e4m3 with DoubleRow perf mode (two fp8 weights
per PE cell -> K=256 per matmul, halving the matmul count); the sigmoid damps
the quantization error so the final rel-err stays ~3e-3 (gate is 2e-2).

Layout trick: H1 is permuted s-major on the host (h1' = s*512 + g, where the
original h1 = g*8 + s).  fc1 then produces hT' tiles [128 h1' partitions x 512
rows]; the grouped fc2 contraction over s becomes 8 fused multiply-accumulate
ops on the vector engine with per-partition scalars (W2 columns), and fc3 is a
plain bf16 matmul over the 512 groups.  Biases are per-partition [128,1]
columns fused into ScalarE sigmoids / a VectorE add.
"""

import sys

for _p in ("/opt/trn_rl_repo",):
    if _p not in sys.path:
        sys.path.append(_p)

import numpy as np
import ml_dtypes

BF16 = ml_dtypes.bfloat16
FP8 = ml_dtypes.float8_e4m3  # == mybir.dt.float8e4

D_IN, H1, H2, D_OUT = 1024, 4096, 512, 256
GS = H1 // H2  # 8
B = 16384
N_CORES = 8
B_SHARD = B // N_CORES  # 2048
NBLK = 512  # rows per block (one PSUM bank of fp32)
NB = B_SHARD // NBLK  # 4
KC = D_IN // 128  # 8 contraction subtiles for fc1
KP = KC // 2  # 4 DoubleRow pairs
CC = H1 // 128  # 32 h1' chunks
NT = H2 // 128  # 4 x2T tiles
ND = D_OUT // 128  # 2 output chunks

_compiled = {}


def _build_nc():
    from concourse import bacc, tile, mybir

    f32 = mybir.dt.float32
    bf16 = mybir.dt.bfloat16
    fp8 = mybir.dt.float8e4
    AF = mybir.ActivationFunctionType
    ALU = mybir.AluOpType
    DR = mybir.MatmulPerfMode.DoubleRow

    nc = bacc.Bacc("TRN2", target_bir_lowering=False, debug=False,
                   num_devices=N_CORES)

    xq = nc.dram_tensor("xq", [128, KC, B_SHARD], fp8, kind="ExternalInput")
    w1q = nc.dram_tensor("w1q", [128, KC, H1], fp8, kind="ExternalInput")
    w2c = nc.dram_tensor("w2c", [128, CC], f32, kind="ExternalInput")
    b1c = nc.dram_tensor("b1c", [128, CC], f32, kind="ExternalInput")
    b2c = nc.dram_tensor("b2c", [128, NT], f32, kind="ExternalInput")
    w3t = nc.dram_tensor("w3t", [H2, D_OUT], bf16, kind="ExternalInput")
    b3c = nc.dram_tensor("b3c", [128, ND], f32, kind="ExternalInput")
    out = nc.dram_tensor("out", [D_OUT, B_SHARD], f32, kind="ExternalOutput")

    with tile.TileContext(nc) as tc:
        with (
            tc.tile_pool(name="wpool", bufs=1) as wpool,
            tc.tile_pool(name="xpool", bufs=1) as xpool,
            tc.tile_pool(name="hpool", bufs=8) as hpool,
            tc.tile_pool(name="accpool", bufs=1) as accpool,
            tc.tile_pool(name="x2pool", bufs=1) as x2pool,
            tc.tile_pool(name="opool", bufs=4) as opool,
            tc.tile_pool(name="psum_h", bufs=8, space="PSUM") as psum_h_pool,
        ):
            psum_o_pool = psum_h_pool
            # --- constants ---
            w2_sb = wpool.tile([128, CC], f32, tag="w2c")
            nc.sync.dma_start(w2_sb[:], w2c.ap()[:])
            b1_sb = wpool.tile([128, CC], f32, tag="b1c")
            nc.sync.dma_start(b1_sb[:], b1c.ap()[:])
            b2_sb = wpool.tile([128, NT], f32, tag="b2c")
            nc.sync.dma_start(b2_sb[:], b2c.ap()[:])
            b3_sb = wpool.tile([128, ND], f32, tag="b3c")
            nc.sync.dma_start(b3_sb[:], b3c.ap()[:])

            # --- fc1 weights / inputs. DMA order matters for the ramp: the
            # first matmuls need only x pair 0 and the first H1 columns of
            # W1 pair 0, so land those first in small pieces. ---
            w1_sb = [None] * KP
            x_sb = [None] * NB
            w1_sb[0] = wpool.tile([128, 2, H1], fp8, tag="w1_0",
                                  name="w1sb_0")
            # first c-chunks of pair 0 in H1 quarters
            for q in range(4):
                nc.sync.dma_start(
                    w1_sb[0][:, :, q * (H1 // 4):(q + 1) * (H1 // 4)],
                    w1q.ap()[:, 0:2, q * (H1 // 4):(q + 1) * (H1 // 4)])
                if q == 0:
                    for n in range(NB):
                        x_sb[n] = xpool.tile([128, KC, NBLK], fp8,
                                             tag=f"x_{n}", name=f"xsb_{n}")
                        nc.sync.dma_start(
                            x_sb[n][:, 0:2, :],
                            xq.ap()[:, 0:2, n * NBLK:(n + 1) * NBLK])
            for j in range(1, KP):
                w1_sb[j] = wpool.tile([128, 2, H1], fp8, tag=f"w1_{j}",
                                      name=f"w1sb_{j}")
                nc.sync.dma_start(w1_sb[j][:],
                                  w1q.ap()[:, 2 * j:2 * j + 2, :])
                for n in range(NB):
                    nc.sync.dma_start(
                        x_sb[n][:, 2 * j:2 * j + 2, :],
                        xq.ap()[:, 2 * j:2 * j + 2,
                                n * NBLK:(n + 1) * NBLK])
            w3_sb = []
            for t_i in range(NT):
                t = wpool.tile([128, D_OUT], bf16, tag=f"w3_{t_i}",
                               name=f"w3sb_{t_i}")
                nc.sync.dma_start(t[:], w3t.ap()[128 * t_i:128 * (t_i + 1), :])
                w3_sb.append(t)

            # fc2 accumulators, one per (row-block, x2 tile)
            acc = [[None] * NT for _ in range(NB)]

            x2_sb = [[None] * NT for _ in range(NB)]

            def fc2_step(c, n, ht):
                t_i = c % NT
                if c < NT:
                    acc[n][t_i] = accpool.tile([128, NBLK], bf16,
                                               tag=f"acc_{n}_{t_i}",
                                               name=f"acc_{n}_{t_i}")
                    nc.vector.tensor_scalar_mul(acc[n][t_i][:], ht[:],
                                                w2_sb[:, c:c + 1])
                else:
                    nc.vector.scalar_tensor_tensor(
                        acc[n][t_i][:], ht[:], w2_sb[:, c:c + 1],
                        acc[n][t_i][:], op0=ALU.mult, op1=ALU.add)
                if c >= CC - NT:
                    # chain for tile t_i is complete -> fc2 sigmoid now so
                    # fc3's t-outer matmuls can start before the last chain
                    t = x2pool.tile([128, NBLK], bf16, tag=f"x2_{n}_{t_i}",
                                    name=f"x2sb_{n}_{t_i}")
                    nc.scalar.activation(t[:], acc[n][t_i][:], AF.Sigmoid,
                                         bias=b2_sb[:, t_i:t_i + 1])
                    x2_sb[n][t_i] = t

            def epilogue(c, ph):
                for n in range(NB):
                    ht = hpool.tile([128, NBLK], bf16, tag="ht",
                                    name=f"ht_{n}_{c}")
                    nc.scalar.activation(ht[:], ph[n][:], AF.Sigmoid,
                                         bias=b1_sb[:, c:c + 1])
                    fc2_step(c, n, ht)

            # --- fc1 + fc2 ---
            # Ramp phase: first 8 c-chunks iterate j-OUTER so the PE only
            # ever needs the W1/x pair that has already DMA'd in, never
            # stalling on the tail of the 6MB weight/input stream.
            RAMP = 8
            ph_ramp = [None] * RAMP
            for c in range(RAMP):
                ph_ramp[c] = [None] * NB
                for n in range(NB):
                    ph_ramp[c][n] = psum_h_pool.tile(
                        [128, NBLK], f32, tag="psum_h", name=f"ph_{n}_{c}")
            for j in range(KP):
                for c in range(RAMP):
                    for n in range(NB):
                        nc.tensor.matmul(
                            ph_ramp[c][n][:],
                            lhsT=w1_sb[j][:, :, 128 * c:128 * (c + 1)],
                            rhs=x_sb[n][:, 2 * j:2 * j + 2, :],
                            start=(j == 0),
                            stop=(j == KP - 1),
                            perf_mode=DR,
                        )
            for c in range(RAMP):
                epilogue(c, ph_ramp[c])

            # Steady phase: c-outer, all weights resident.
            for c in range(RAMP, CC):
                ph = [None] * NB
                for n in range(NB):
                    ph[n] = psum_h_pool.tile([128, NBLK], f32, tag="psum_h",
                                             name=f"ph_{n}_{c}")
                for j in range(KP):
                    for n in range(NB):
                        nc.tensor.matmul(
                            ph[n][:],
                            lhsT=w1_sb[j][:, :, 128 * c:128 * (c + 1)],
                            rhs=x_sb[n][:, 2 * j:2 * j + 2, :],
                            start=(j == 0),
                            stop=(j == KP - 1),
                            perf_mode=DR,
                        )
                epilogue(c, ph)

            # --- fc3: t-outer so t=0..2 matmuls overlap the tail of the
            # fc2 sigmoid chain; accumulation start on t=0, stop on t=3 ---
            po = [[None] * ND for _ in range(NB)]
            for n in range(NB):
                for d in range(ND):
                    po[n][d] = psum_o_pool.tile([128, NBLK], f32,
                                                tag="psum_h",
                                                name=f"po_{n}_{d}")
            for t_i in range(NT):
                for n in range(NB):
                    for d in range(ND):
                        nc.tensor.matmul(
                            po[n][d][:],
                            lhsT=w3_sb[t_i][:, 128 * d:128 * (d + 1)],
                            rhs=x2_sb[n][t_i][:],
                            start=(t_i == 0),
                            stop=(t_i == NT - 1),
                        )
            for n in range(NB):
                for d in range(ND):
                    ot = opool.tile([128, NBLK], f32, tag="ot",
                                    name=f"ot_{n}_{d}")
                    nc.vector.tensor_scalar_add(ot[:], po[n][d][:],
                                                b3_sb[:, d:d + 1])
                    nc.sync.dma_start(
                        out.ap()[128 * d:128 * (d + 1),
                                 n * NBLK:(n + 1) * NBLK], ot[:])

    nc.compile()
    return nc


def get_nc():
    if "nc" not in _compiled:
        _compiled["nc"] = _build_nc()
    return _compiled["nc"]


def make_in_maps(x, W1, b1, W2, b2, W3, b3):
    x = np.asarray(x, dtype=np.float32)
    W1 = np.asarray(W1, dtype=np.float32)
    b1 = np.asarray(b1, dtype=np.float32)
    W2 = np.asarray(W2, dtype=np.float32)
    b2 = np.asarray(b2, dtype=np.float32)
    W3 = np.asarray(W3, dtype=np.float32)
    b3 = np.asarray(b3, dtype=np.float32)

    # s-major permutation of H1: new index p = s*H2 + g  (old h1 = g*GS + s)
    p = np.arange(H1)
    perm = (p % H2) * GS + (p // H2)
    W1p = W1[perm, :]
    b1p = b1[perm]

    # fp8 fc1 operands in DoubleRow layout [128, KC, *]:
    # element (p, j, m) holds contraction index k = 128*j + p
    w1t = W1p.T.astype(FP8)  # [D_IN, H1]
    w1q_h = np.ascontiguousarray(
        w1t.reshape(KC, 128, H1).transpose(1, 0, 2))
    xt = x.T.astype(FP8)  # [D_IN, B]
    xq_h = np.ascontiguousarray(
        xt.reshape(KC, 128, B).transpose(1, 0, 2))

    b1c_h = np.ascontiguousarray(b1p.reshape(CC, 128).T, dtype=np.float32)
    # chunk c: s = c//NT, tile t = c%NT, partition k <-> group 128*t + k
    w2c_h = np.empty((128, CC), dtype=np.float32)
    for c in range(CC):
        w2c_h[:, c] = W2[128 * (c % NT):128 * (c % NT) + 128, c // NT]
    b2c_h = np.ascontiguousarray(b2.reshape(NT, 128).T, dtype=np.float32)
    w3t_h = np.ascontiguousarray(W3.T).astype(BF16)  # [H2, D_OUT]
    b3c_h = np.ascontiguousarray(b3.reshape(ND, 128).T, dtype=np.float32)

    in_maps = []
    for i in range(N_CORES):
        in_maps.append({
            "xq": np.ascontiguousarray(
                xq_h[:, :, i * B_SHARD:(i + 1) * B_SHARD]),
            "w1q": w1q_h,
            "w2c": w2c_h,
            "b1c": b1c_h,
            "b2c": b2c_h,
            "w3t": w3t_h,
            "b3c": b3c_h,
        })
    return in_maps


def kernel(x, W1, b1, W2, b2, W3, b3):
    from concourse.bass_utils import run_bass_kernel_spmd

    nc = get_nc()
    in_maps = make_in_maps(x, W1, b1, W2, b2, W3, b3)
    res = run_bass_kernel_spmd(nc, in_maps, core_ids=list(range(N_CORES)))
    outT = np.concatenate([res.results[i]["out"] for i in range(N_CORES)],
                          axis=1)  # [D_OUT, B]
    return np.ascontiguousarray(outT.T)


# revision 15
# speedup vs baseline: 1.1657x; 1.0108x over previous
"""Trainium2 Bass kernel for AdaptiveNet MLP (fc1+sigmoid, grouped fc2+sigmoid, fc3).

Sharding: pure data-parallel over batch across 8 NeuronCores (no collectives).
Each core computes its 2048-row shard through all three layers.

fc1 (95% of FLOPs) runs in fp8-e4m3 with DoubleRow perf mode (two fp8 weights
per PE cell -> K=256 per matmul, halving the matmul count); the sigmoid damps
the quantization error so the final rel-err stays ~3e-3 (gate is 2e-2).

Layout trick: H1 is permuted s-major on the host (h1' = s*512 + g, where the
original h1 = g*8 + s).  fc1 then produces hT' tiles [128 h1' partitions x 512
rows]; the grouped fc2 contraction over s becomes 8 fused multiply-accumulate
ops on the vector engine with per-partition scalars (W2 columns), and fc3 is a
plain bf16 matmul over the 512 groups.  Biases are per-partition [128,1]
columns fused into ScalarE sigmoids / a VectorE add.
"""

import sys

for _p in ("/opt/trn_rl_repo",):
    if _p not in sys.path:
        sys.path.append(_p)

import numpy as np
import ml_dtypes

BF16 = ml_dtypes.bfloat16
FP8 = ml_dtypes.float8_e4m3  # == mybir.dt.float8e4

D_IN, H1, H2, D_OUT = 1024, 4096, 512, 256
GS = H1 // H2  # 8
B = 16384
N_CORES = 8
B_SHARD = B // N_CORES  # 2048
NBLK = 512  # rows per block (one PSUM bank of fp32)
NB = B_SHARD // NBLK  # 4
KC = D_IN // 128  # 8 contraction subtiles for fc1
KP = KC // 2  # 4 DoubleRow pairs
CC = H1 // 128  # 32 h1' chunks
NT = H2 // 128  # 4 x2T tiles
ND = D_OUT // 128  # 2 output chunks

_compiled = {}


def _build_nc():
    from concourse import bacc, tile, mybir

    f32 = mybir.dt.float32
    bf16 = mybir.dt.bfloat16
    fp8 = mybir.dt.float8e4
    AF = mybir.ActivationFunctionType
    ALU = mybir.AluOpType
    DR = mybir.MatmulPerfMode.DoubleRow

    nc = bacc.Bacc("TRN2", target_bir_lowering=False, debug=False,
                   num_devices=N_CORES)

    xq = nc.dram_tensor("xq", [128, KC, B_SHARD], fp8, kind="ExternalInput")
    w1q = nc.dram_tensor("w1q", [128, KC, H1], fp8, kind="ExternalInput")
    w2c = nc.dram_tensor("w2c", [128, CC], f32, kind="ExternalInput")
    b1c = nc.dram_tensor("b1c", [128, CC], f32, kind="ExternalInput")
    b2c = nc.dram_tensor("b2c", [128, NT], f32, kind="ExternalInput")
    w3t = nc.dram_tensor("w3t", [H2, D_OUT], bf16, kind="ExternalInput")
    b3c = nc.dram_tensor("b3c", [128, ND], f32, kind="ExternalInput")
    out = nc.dram_tensor("out", [D_OUT, B_SHARD], f32, kind="ExternalOutput")

    with tile.TileContext(nc) as tc:
        with (
            tc.tile_pool(name="wpool", bufs=1) as wpool,
            tc.tile_pool(name="xpool", bufs=1) as xpool,
            tc.tile_pool(name="hpool", bufs=8) as hpool,
            tc.tile_pool(name="accpool", bufs=1) as accpool,
            tc.tile_pool(name="x2pool", bufs=1) as x2pool,
            tc.tile_pool(name="opool", bufs=4) as opool,
            tc.tile_pool(name="psum_h", bufs=8, space="PSUM") as psum_h_pool,
        ):
            psum_o_pool = psum_h_pool
            # --- fc1 weights / inputs. Spread the big input DMAs across
            # engine queues so they issue (and stream) concurrently; land
            # the data the ramp needs first (W1 pair 0 head, x tiles). ---
            w1_sb = [None] * KP
            for j in range(KP):
                w1_sb[j] = wpool.tile([128, 2, H1], fp8, tag=f"w1_{j}",
                                      name=f"w1sb_{j}")
            x_sb = [None] * NB
            for n in range(NB):
                x_sb[n] = xpool.tile([128, KC, NBLK], fp8,
                                     tag=f"x_{n}", name=f"xsb_{n}")
            H1H = H1 // 2
            # W1 pair 0 on sync, x tiles on gpsimd, later pairs on scalar:
            # three issuing queues so the streams run concurrently
            nc.sync.dma_start(w1_sb[0][:, :, 0:H1H], w1q.ap()[:, 0:2, 0:H1H])
            for n in range(NB):
                nc.gpsimd.dma_start(x_sb[n][:],
                                    xq.ap()[:, :, n * NBLK:(n + 1) * NBLK])
            nc.sync.dma_start(w1_sb[0][:, :, H1H:H1],
                              w1q.ap()[:, 0:2, H1H:H1])
            nc.scalar.dma_start(w1_sb[1][:], w1q.ap()[:, 2:4, :])
            nc.gpsimd.dma_start(w1_sb[2][:], w1q.ap()[:, 4:6, :])
            nc.scalar.dma_start(w1_sb[3][:], w1q.ap()[:, 6:8, :])

            # constants + fc3 weights (tiny)
            w2_sb = wpool.tile([128, CC], f32, tag="w2c")
            nc.sync.dma_start(w2_sb[:], w2c.ap()[:])
            b1_sb = wpool.tile([128, CC], f32, tag="b1c")
            nc.sync.dma_start(b1_sb[:], b1c.ap()[:])
            b2_sb = wpool.tile([128, NT], f32, tag="b2c")
            nc.sync.dma_start(b2_sb[:], b2c.ap()[:])
            b3_sb = wpool.tile([128, ND], f32, tag="b3c")
            nc.sync.dma_start(b3_sb[:], b3c.ap()[:])
            w3_sb = []
            for t_i in range(NT):
                t = wpool.tile([128, D_OUT], bf16, tag=f"w3_{t_i}",
                               name=f"w3sb_{t_i}")
                nc.sync.dma_start(t[:], w3t.ap()[128 * t_i:128 * (t_i + 1), :])
                w3_sb.append(t)

            # fc2 accumulators, one per (row-block, x2 tile)
            acc = [[None] * NT for _ in range(NB)]

            x2_sb = [[None] * NT for _ in range(NB)]

            def fc2_step(c, n, ht):
                t_i = c % NT
                if c < NT:
                    acc[n][t_i] = accpool.tile([128, NBLK], bf16,
                                               tag=f"acc_{n}_{t_i}",
                                               name=f"acc_{n}_{t_i}")
                    nc.vector.tensor_scalar_mul(acc[n][t_i][:], ht[:],
                                                w2_sb[:, c:c + 1])
                else:
                    nc.vector.scalar_tensor_tensor(
                        acc[n][t_i][:], ht[:], w2_sb[:, c:c + 1],
                        acc[n][t_i][:], op0=ALU.mult, op1=ALU.add)
                if c >= CC - NT:
                    # chain for tile t_i is complete -> fc2 sigmoid now so
                    # fc3's t-outer matmuls can start before the last chain
                    t = x2pool.tile([128, NBLK], bf16, tag=f"x2_{n}_{t_i}",
                                    name=f"x2sb_{n}_{t_i}")
                    nc.scalar.activation(t[:], acc[n][t_i][:], AF.Sigmoid,
                                         bias=b2_sb[:, t_i:t_i + 1])
                    x2_sb[n][t_i] = t

            def epilogue(c, ph):
                for n in range(NB):
                    ht = hpool.tile([128, NBLK], bf16, tag="ht",
                                    name=f"ht_{n}_{c}")
                    nc.scalar.activation(ht[:], ph[n][:], AF.Sigmoid,
                                         bias=b1_sb[:, c:c + 1])
                    fc2_step(c, n, ht)

            # --- fc1 + fc2 ---
            # Ramp phase: first 8 c-chunks iterate j-OUTER so the PE only
            # ever needs the W1/x pair that has already DMA'd in, never
            # stalling on the tail of the 6MB weight/input stream.
            RAMP = 8
            ph_ramp = [None] * RAMP
            for c in range(RAMP):
                ph_ramp[c] = [None] * NB
                for n in range(NB):
                    ph_ramp[c][n] = psum_h_pool.tile(
                        [128, NBLK], f32, tag="psum_h", name=f"ph_{n}_{c}")
            for j in range(KP):
                for c in range(RAMP):
                    for n in range(NB):
                        nc.tensor.matmul(
                            ph_ramp[c][n][:],
                            lhsT=w1_sb[j][:, :, 128 * c:128 * (c + 1)],
                            rhs=x_sb[n][:, 2 * j:2 * j + 2, :],
                            start=(j == 0),
                            stop=(j == KP - 1),
                            perf_mode=DR,
                        )
            for c in range(RAMP):
                epilogue(c, ph_ramp[c])

            # Steady phase: c-outer, all weights resident.
            for c in range(RAMP, CC):
                ph = [None] * NB
                for n in range(NB):
                    ph[n] = psum_h_pool.tile([128, NBLK], f32, tag="psum_h",
                                             name=f"ph_{n}_{c}")
                for j in range(KP):
                    for n in range(NB):
                        nc.tensor.matmul(
                            ph[n][:],
                            lhsT=w1_sb[j][:, :, 128 * c:128 * (c + 1)],
                            rhs=x_sb[n][:, 2 * j:2 * j + 2, :],
                            start=(j == 0),
                            stop=(j == KP - 1),
                            perf_mode=DR,
                        )
                epilogue(c, ph)

            # --- fc3: n-outer so each row-block's output DVE/DMA overlaps
            # the next row-block's matmuls ---
            for n in range(NB):
                for d in range(ND):
                    po = psum_o_pool.tile([128, NBLK], f32, tag="psum_h",
                                          name=f"po_{n}_{d}")
                    for t_i in range(NT):
                        nc.tensor.matmul(
                            po[:],
                            lhsT=w3_sb[t_i][:, 128 * d:128 * (d + 1)],
                            rhs=x2_sb[n][t_i][:],
                            start=(t_i == 0),
                            stop=(t_i == NT - 1),
                        )
                    ot = opool.tile([128, NBLK], f32, tag="ot",
                                    name=f"ot_{n}_{d}")
                    nc.vector.tensor_scalar_add(ot[:], po[:],
                                                b3_sb[:, d:d + 1])
                    nc.sync.dma_start(
                        out.ap()[128 * d:128 * (d + 1),
                                 n * NBLK:(n + 1) * NBLK], ot[:])

    nc.compile()
    return nc


def get_nc():
    if "nc" not in _compiled:
        _compiled["nc"] = _build_nc()
    return _compiled["nc"]


def make_in_maps(x, W1, b1, W2, b2, W3, b3):
    x = np.asarray(x, dtype=np.float32)
    W1 = np.asarray(W1, dtype=np.float32)
    b1 = np.asarray(b1, dtype=np.float32)
    W2 = np.asarray(W2, dtype=np.float32)
    b2 = np.asarray(b2, dtype=np.float32)
    W3 = np.asarray(W3, dtype=np.float32)
    b3 = np.asarray(b3, dtype=np.float32)

    # s-major permutation of H1: new index p = s*H2 + g  (old h1 = g*GS + s)
    p = np.arange(H1)
    perm = (p % H2) * GS + (p // H2)
    W1p = W1[perm, :]
    b1p = b1[perm]

    # fp8 fc1 operands in DoubleRow layout [128, KC, *]:
    # element (p, j, m) holds contraction index k = 128*j + p
    w1t = W1p.T.astype(FP8)  # [D_IN, H1]
    w1q_h = np.ascontiguousarray(
        w1t.reshape(KC, 128, H1).transpose(1, 0, 2))
    xt = x.T.astype(FP8)  # [D_IN, B]
    xq_h = np.ascontiguousarray(
        xt.reshape(KC, 128, B).transpose(1, 0, 2))

    b1c_h = np.ascontiguousarray(b1p.reshape(CC, 128).T, dtype=np.float32)
    # chunk c: s = c//NT, tile t = c%NT, partition k <-> group 128*t + k
    w2c_h = np.empty((128, CC), dtype=np.float32)
    for c in range(CC):
        w2c_h[:, c] = W2[128 * (c % NT):128 * (c % NT) + 128, c // NT]
    b2c_h = np.ascontiguousarray(b2.reshape(NT, 128).T, dtype=np.float32)
    w3t_h = np.ascontiguousarray(W3.T).astype(BF16)  # [H2, D_OUT]
    b3c_h = np.ascontiguousarray(b3.reshape(ND, 128).T, dtype=np.float32)

    in_maps = []
    for i in range(N_CORES):
        in_maps.append({
            "xq": np.ascontiguousarray(
                xq_h[:, :, i * B_SHARD:(i + 1) * B_SHARD]),
            "w1q": w1q_h,
            "w2c": w2c_h,
            "b1c": b1c_h,
            "b2c": b2c_h,
            "w3t": w3t_h,
            "b3c": b3c_h,
        })
    return in_maps


def kernel(x, W1, b1, W2, b2, W3, b3):
    from concourse.bass_utils import run_bass_kernel_spmd

    nc = get_nc()
    in_maps = make_in_maps(x, W1, b1, W2, b2, W3, b3)
    res = run_bass_kernel_spmd(nc, in_maps, core_ids=list(range(N_CORES)))
    outT = np.concatenate([res.results[i]["out"] for i in range(N_CORES)],
                          axis=1)  # [D_OUT, B]
    return np.ascontiguousarray(outT.T)


# revision 16
# speedup vs baseline: 1.1779x; 1.0105x over previous
"""Trainium2 Bass kernel for AdaptiveNet MLP (fc1+sigmoid, grouped fc2+sigmoid, fc3).

Sharding: pure data-parallel over batch across 8 NeuronCores (no collectives).
Each core computes its 2048-row shard through all three layers.

fc1 (95% of FLOPs) runs in fp8-e4m3 with DoubleRow perf mode (two fp8 weights
per PE cell -> K=256 per matmul, halving the matmul count); the sigmoid damps
the quantization error so the final rel-err stays ~3e-3 (gate is 2e-2).

Layout trick: H1 is permuted s-major on the host (h1' = s*512 + g, where the
original h1 = g*8 + s).  fc1 then produces hT' tiles [128 h1' partitions x 512
rows]; the grouped fc2 contraction over s becomes 8 fused multiply-accumulate
ops on the vector engine with per-partition scalars (W2 columns), and fc3 is a
plain bf16 matmul over the 512 groups.  Biases are per-partition [128,1]
columns fused into ScalarE sigmoids / a VectorE add.
"""

import sys

for _p in ("/opt/trn_rl_repo",):
    if _p not in sys.path:
        sys.path.append(_p)

import numpy as np
import ml_dtypes

BF16 = ml_dtypes.bfloat16
FP8 = ml_dtypes.float8_e4m3  # == mybir.dt.float8e4

D_IN, H1, H2, D_OUT = 1024, 4096, 512, 256
GS = H1 // H2  # 8
B = 16384
N_CORES = 8
B_SHARD = B // N_CORES  # 2048
NBLK = 512  # rows per block (one PSUM bank of fp32)
NB = B_SHARD // NBLK  # 4
KC = D_IN // 128  # 8 contraction subtiles for fc1
KP = KC // 2  # 4 DoubleRow pairs
CC = H1 // 128  # 32 h1' chunks
NT = H2 // 128  # 4 x2T tiles
ND = D_OUT // 128  # 2 output chunks

_compiled = {}


def _build_nc():
    from concourse import bacc, tile, mybir

    f32 = mybir.dt.float32
    bf16 = mybir.dt.bfloat16
    fp8 = mybir.dt.float8e4
    AF = mybir.ActivationFunctionType
    ALU = mybir.AluOpType
    DR = mybir.MatmulPerfMode.DoubleRow

    nc = bacc.Bacc("TRN2", target_bir_lowering=False, debug=False,
                   num_devices=N_CORES)

    xq = nc.dram_tensor("xq", [128, KC, B_SHARD], fp8, kind="ExternalInput")
    w1q = nc.dram_tensor("w1q", [128, KC, H1], fp8, kind="ExternalInput")
    w2c = nc.dram_tensor("w2c", [128, CC], f32, kind="ExternalInput")
    b1c = nc.dram_tensor("b1c", [128, CC], f32, kind="ExternalInput")
    b2c = nc.dram_tensor("b2c", [128, NT], f32, kind="ExternalInput")
    w3t = nc.dram_tensor("w3t", [H2, D_OUT], bf16, kind="ExternalInput")
    b3c = nc.dram_tensor("b3c", [128, ND], f32, kind="ExternalInput")
    out = nc.dram_tensor("out", [D_OUT, B_SHARD], f32, kind="ExternalOutput")

    with tile.TileContext(nc) as tc:
        with (
            tc.tile_pool(name="wpool", bufs=1) as wpool,
            tc.tile_pool(name="xpool", bufs=1) as xpool,
            tc.tile_pool(name="hpool", bufs=8) as hpool,
            tc.tile_pool(name="accpool", bufs=1) as accpool,
            tc.tile_pool(name="x2pool", bufs=1) as x2pool,
            tc.tile_pool(name="opool", bufs=4) as opool,
            tc.tile_pool(name="psum_h", bufs=8, space="PSUM") as psum_h_pool,
        ):
            psum_o_pool = psum_h_pool
            # --- fc1 weights / inputs. Spread the big input DMAs across
            # engine queues so they issue (and stream) concurrently; land
            # the data the ramp needs first (W1 pair 0 head, x tiles). ---
            w1_sb = [None] * KP
            for j in range(KP):
                w1_sb[j] = wpool.tile([128, 2, H1], fp8, tag=f"w1_{j}",
                                      name=f"w1sb_{j}")
            x_sb = [None] * NB
            for n in range(NB):
                x_sb[n] = xpool.tile([128, KC, NBLK], fp8,
                                     tag=f"x_{n}", name=f"xsb_{n}")
            # Three issuing queues (sync/scalar HWDGE, gpsimd SWDGE), FIFO
            # within each.  Wave 1 = exactly what ramp phase j=0 touches
            # (W1 pair 0 cols 0:1024, x pair-0 slices); wave 2 = the front
            # columns of the later pairs + remaining x; wave 3 = the rest.
            RAMP = 8
            RC = RAMP * 128  # H1 columns touched by the ramp phase

            def wslice(j, c0, c1):
                return (w1_sb[j][:, :, c0:c1], w1q.ap()[:, 2 * j:2 * j + 2,
                                                        c0:c1])

            def xslice(n, p0, p1):
                return (x_sb[n][:, p0:p1, :],
                        xq.ap()[:, p0:p1, n * NBLK:(n + 1) * NBLK])

            # wave 1
            nc.sync.dma_start(*wslice(0, 0, RC))
            nc.scalar.dma_start(*xslice(0, 0, 2))
            nc.gpsimd.dma_start(*xslice(1, 0, 2))
            nc.scalar.dma_start(*xslice(2, 0, 2))
            nc.gpsimd.dma_start(*xslice(3, 0, 2))
            # wave 2: ramp fronts of pairs 1..3, x tails, b1/w2 consts
            b1_sb = wpool.tile([128, CC], f32, tag="b1c")
            w2_sb = wpool.tile([128, CC], f32, tag="w2c")
            nc.sync.dma_start(*wslice(1, 0, RC))
            nc.scalar.dma_start(*xslice(0, 2, KC))
            nc.gpsimd.dma_start(*xslice(1, 2, KC))
            nc.sync.dma_start(*wslice(2, 0, RC))
            nc.scalar.dma_start(*xslice(2, 2, KC))
            nc.gpsimd.dma_start(*xslice(3, 2, KC))
            nc.sync.dma_start(*wslice(3, 0, RC))
            nc.sync.dma_start(b1_sb[:], b1c.ap()[:])
            nc.sync.dma_start(w2_sb[:], w2c.ap()[:])
            # wave 3: steady-state columns of all pairs + small tail consts
            nc.sync.dma_start(*wslice(0, RC, H1))
            nc.scalar.dma_start(*wslice(1, RC, H1))
            nc.gpsimd.dma_start(*wslice(2, RC, H1))
            nc.sync.dma_start(*wslice(3, RC, H1))
            b2_sb = wpool.tile([128, NT], f32, tag="b2c")
            nc.scalar.dma_start(b2_sb[:], b2c.ap()[:])
            b3_sb = wpool.tile([128, ND], f32, tag="b3c")
            nc.gpsimd.dma_start(b3_sb[:], b3c.ap()[:])
            w3_sb = []
            for t_i in range(NT):
                t = wpool.tile([128, D_OUT], bf16, tag=f"w3_{t_i}",
                               name=f"w3sb_{t_i}")
                nc.gpsimd.dma_start(t[:],
                                    w3t.ap()[128 * t_i:128 * (t_i + 1), :])
                w3_sb.append(t)

            # fc2 accumulators, one per (row-block, x2 tile)
            acc = [[None] * NT for _ in range(NB)]

            x2_sb = [[None] * NT for _ in range(NB)]

            def fc2_step(c, n, ht):
                t_i = c % NT
                if c < NT:
                    acc[n][t_i] = accpool.tile([128, NBLK], bf16,
                                               tag=f"acc_{n}_{t_i}",
                                               name=f"acc_{n}_{t_i}")
                    nc.vector.tensor_scalar_mul(acc[n][t_i][:], ht[:],
                                                w2_sb[:, c:c + 1])
                else:
                    nc.vector.scalar_tensor_tensor(
                        acc[n][t_i][:], ht[:], w2_sb[:, c:c + 1],
                        acc[n][t_i][:], op0=ALU.mult, op1=ALU.add)
                if c >= CC - NT:
                    # chain for tile t_i is complete -> fc2 sigmoid now so
                    # fc3's t-outer matmuls can start before the last chain
                    t = x2pool.tile([128, NBLK], bf16, tag=f"x2_{n}_{t_i}",
                                    name=f"x2sb_{n}_{t_i}")
                    nc.scalar.activation(t[:], acc[n][t_i][:], AF.Sigmoid,
                                         bias=b2_sb[:, t_i:t_i + 1])
                    x2_sb[n][t_i] = t

            def epilogue(c, ph):
                for n in range(NB):
                    ht = hpool.tile([128, NBLK], bf16, tag="ht",
                                    name=f"ht_{n}_{c}")
                    nc.scalar.activation(ht[:], ph[n][:], AF.Sigmoid,
                                         bias=b1_sb[:, c:c + 1])
                    fc2_step(c, n, ht)

            # --- fc1 + fc2 ---
            # Ramp phase: first 8 c-chunks iterate j-OUTER so the PE only
            # ever needs the W1/x pair that has already DMA'd in, never
            # stalling on the tail of the 6MB weight/input stream.
            RAMP = 8
            ph_ramp = [None] * RAMP
            for c in range(RAMP):
                ph_ramp[c] = [None] * NB
                for n in range(NB):
                    ph_ramp[c][n] = psum_h_pool.tile(
                        [128, NBLK], f32, tag="psum_h", name=f"ph_{n}_{c}")
            for j in range(KP):
                for c in range(RAMP):
                    for n in range(NB):
                        nc.tensor.matmul(
                            ph_ramp[c][n][:],
                            lhsT=w1_sb[j][:, :, 128 * c:128 * (c + 1)],
                            rhs=x_sb[n][:, 2 * j:2 * j + 2, :],
                            start=(j == 0),
                            stop=(j == KP - 1),
                            perf_mode=DR,
                        )
            for c in range(RAMP):
                epilogue(c, ph_ramp[c])

            # Steady phase: c-outer, all weights resident.
            for c in range(RAMP, CC):
                ph = [None] * NB
                for n in range(NB):
                    ph[n] = psum_h_pool.tile([128, NBLK], f32, tag="psum_h",
                                             name=f"ph_{n}_{c}")
                for j in range(KP):
                    for n in range(NB):
                        nc.tensor.matmul(
                            ph[n][:],
                            lhsT=w1_sb[j][:, :, 128 * c:128 * (c + 1)],
                            rhs=x_sb[n][:, 2 * j:2 * j + 2, :],
                            start=(j == 0),
                            stop=(j == KP - 1),
                            perf_mode=DR,
                        )
                epilogue(c, ph)

            # --- fc3: n-outer so each row-block's output DVE/DMA overlaps
            # the next row-block's matmuls ---
            for n in range(NB):
                for d in range(ND):
                    po = psum_o_pool.tile([128, NBLK], f32, tag="psum_h",
                                          name=f"po_{n}_{d}")
                    for t_i in range(NT):
                        nc.tensor.matmul(
                            po[:],
                            lhsT=w3_sb[t_i][:, 128 * d:128 * (d + 1)],
                            rhs=x2_sb[n][t_i][:],
                            start=(t_i == 0),
                            stop=(t_i == NT - 1),
                        )
                    ot = opool.tile([128, NBLK], f32, tag="ot",
                                    name=f"ot_{n}_{d}")
                    nc.vector.tensor_scalar_add(ot[:], po[:],
                                                b3_sb[:, d:d + 1])
                    nc.sync.dma_start(
                        out.ap()[128 * d:128 * (d + 1),
                                 n * NBLK:(n + 1) * NBLK], ot[:])

    nc.compile()
    return nc


def get_nc():
    if "nc" not in _compiled:
        _compiled["nc"] = _build_nc()
    return _compiled["nc"]


def make_in_maps(x, W1, b1, W2, b2, W3, b3):
    x = np.asarray(x, dtype=np.float32)
    W1 = np.asarray(W1, dtype=np.float32)
    b1 = np.asarray(b1, dtype=np.float32)
    W2 = np.asarray(W2, dtype=np.float32)
    b2 = np.asarray(b2, dtype=np.float32)
    W3 = np.asarray(W3, dtype=np.float32)
    b3 = np.asarray(b3, dtype=np.float32)

    # s-major permutation of H1: new index p = s*H2 + g  (old h1 = g*GS + s)
    p = np.arange(H1)
    perm = (p % H2) * GS + (p // H2)
    W1p = W1[perm, :]
    b1p = b1[perm]

    # fp8 fc1 operands in DoubleRow layout [128, KC, *]:
    # element (p, j, m) holds contraction index k = 128*j + p
    w1t = W1p.T.astype(FP8)  # [D_IN, H1]
    w1q_h = np.ascontiguousarray(
        w1t.reshape(KC, 128, H1).transpose(1, 0, 2))
    xt = x.T.astype(FP8)  # [D_IN, B]
    xq_h = np.ascontiguousarray(
        xt.reshape(KC, 128, B).transpose(1, 0, 2))

    b1c_h = np.ascontiguousarray(b1p.reshape(CC, 128).T, dtype=np.float32)
    # chunk c: s = c//NT, tile t = c%NT, partition k <-> group 128*t + k
    w2c_h = np.empty((128, CC), dtype=np.float32)
    for c in range(CC):
        w2c_h[:, c] = W2[128 * (c % NT):128 * (c % NT) + 128, c // NT]
    b2c_h = np.ascontiguousarray(b2.reshape(NT, 128).T, dtype=np.float32)
    w3t_h = np.ascontiguousarray(W3.T).astype(BF16)  # [H2, D_OUT]
    b3c_h = np.ascontiguousarray(b3.reshape(ND, 128).T, dtype=np.float32)

    in_maps = []
    for i in range(N_CORES):
        in_maps.append({
            "xq": np.ascontiguousarray(
                xq_h[:, :, i * B_SHARD:(i + 1) * B_SHARD]),
            "w1q": w1q_h,
            "w2c": w2c_h,
            "b1c": b1c_h,
            "b2c": b2c_h,
            "w3t": w3t_h,
            "b3c": b3c_h,
        })
    return in_maps


def kernel(x, W1, b1, W2, b2, W3, b3):
    from concourse.bass_utils import run_bass_kernel_spmd

    nc = get_nc()
    in_maps = make_in_maps(x, W1, b1, W2, b2, W3, b3)
    res = run_bass_kernel_spmd(nc, in_maps, core_ids=list(range(N_CORES)))
    outT = np.concatenate([res.results[i]["out"] for i in range(N_CORES)],
                          axis=1)  # [D_OUT, B]
    return np.ascontiguousarray(outT.T)
